# revision 1
# baseline (speedup 1.0000x reference)
"""Bass/Tile TRN2 kernel for nn_BertAttention (B=2, S=4096, H=768) on 8 NeuronCores.

Sharding: core c handles batch b = c // 4, query chunk qc = c % 4 (1024 queries).
Each core computes K/V projections for its full batch (4x redundant), attention
for its own 1024 queries, then Wo1 + LN1 + Wo2 + LN2 token-parallel.

Perf structure (v3):
- All matmuls except Wo2 run in fp8e4 with DoubleRow perf mode. Weights are
  host-prescaled x16 into fp8's normal range; the 1/16 is folded into the
  PSUM->SBUF copy scale (projections) or the softmax denominator (Wo1 path).
- The attention mask is folded into the K-projection input on the host
  (K_j = Wk @ (m_j x_j)), so exp needs only constant scale/bias.
- exp uses bias -2.5; the unnormalized ctx is stored fp8 at 1/8 scale so it
  stays below fp8e4's 240 max; the den matmul constant 2.0 = 16/8 makes
  rs = 1/den normalize the Wo1 PSUM exactly.
- PV accumulates ctx^T[h, q] directly (lhsT = V[k, h-slice], rhs = P[k, q]):
  no ctx transpose. Softmax normalization is deferred to the Wo1 PSUM where
  queries are the partition dim (per-partition scale). Denominators come from
  tiny pt @ const matmuls.
- Software pipeline: scores/exp for block i run interleaved with PV for block
  i-1 (hc-major, so 3 PSUM banks hold 6 accumulators) and the transpose/Wo2/
  LN2/store tail of block i-2; the Wo1/LN1 tail of block i-1 is emitted after
  the slots.
- K/V/Q projections stream interleaved per 512-key block so the PSUM->SBUF
  copies spread across DVE (K) and ACT (V/Q) concurrently.
- rstd = exp(-0.5 ln(H var + H eps) + 0.5 ln H) keeps everything in the
  ln/exp activation table (no table reloads).
"""

import sys

if "/opt/trn_rl_repo" not in sys.path:
    sys.path.insert(0, "/opt/trn_rl_repo")

import numpy as np
import ml_dtypes

import concourse.bass as bass
import concourse.mybir as mybir
import concourse.tile as tile
from concourse import bacc
from concourse.masks import make_identity

BF16 = mybir.dt.bfloat16
F32 = mybir.dt.float32
F8 = mybir.dt.float8e4
DR = mybir.MatmulPerfMode.DoubleRow

B, S, H = 2, 4096, 768
NQ = S // 4          # queries per core
HC = H // 128        # 6 hidden chunks
HP = HC // 2         # 3 hidden chunk pairs (DoubleRow)
KC = S // 128        # 32 key chunks
KP = KC // 2         # 16 key chunk pairs
QB = 256             # query block
NQB = NQ // QB       # 4 query blocks
EPS = 1e-12
NCORES = 8
INV16 = 1.0 / 16.0
EXP_SCALE = 1.0 / float(np.sqrt(H))
EXP_BIAS = -2.5


def _emit(nc, tc, io, zero_qkbias=True, zero_affine2=True, ones_mask=True):
    import os
    PHASE = int(os.environ.get("KERNEL_PHASE", "9"))
    TB0 = int(os.environ.get("KERNEL_TB0", "6"))
    PVD = int(os.environ.get("KERNEL_PVD", "7"))
    ZERO_QKBIAS = zero_qkbias
    ZERO_AFFINE2 = zero_affine2
    (xTk, xTv, xqT, wqT, wkT, wvT, wo1T, wo2T, bq, bk, g2v, be2v,
     xb1, xb2, out) = io

    from contextlib import ExitStack
    ctx_mgr = ExitStack()
    consts = ctx_mgr.enter_context(tc.tile_pool(name="consts", bufs=1))
    wpool = ctx_mgr.enter_context(tc.tile_pool(name="wpool", bufs=1))
    kvq = ctx_mgr.enter_context(tc.tile_pool(name="kvq", bufs=1))
    xtp = ctx_mgr.enter_context(tc.tile_pool(name="xtp", bufs=3))
    ptp = ctx_mgr.enter_context(tc.tile_pool(name="ptp", bufs=2))
    tailp = ctx_mgr.enter_context(tc.tile_pool(name="tailp", bufs=3))
    smallp = ctx_mgr.enter_context(tc.tile_pool(name="smallp", bufs=4))

    # ---- constants ----
    ident = consts.tile([128, 128], BF16, tag="ident")
    make_identity(nc, ident)

    bq_sb = consts.tile([128, HC], F32, tag="bq")
    bk_sb = consts.tile([128, HC], F32, tag="bk")
    nc.gpsimd.dma_start(out=bq_sb, in_=bq.ap().rearrange("(c p) -> p c", p=128))
    nc.gpsimd.dma_start(out=bk_sb, in_=bk.ap().rearrange("(c p) -> p c", p=128))

    # den constant: 2.0 = 16 (Wo1 host prescale) / 8 (ctx fp8 downscale), so
    # rs = 1/(2*sum(p)) exactly normalizes the Wo1 PSUM = 2 * ctxU @ Wo1.
    ones16 = consts.tile([128, 2, 1], F8, tag="ones16")
    nc.vector.memset(ones16, 2.0)

    c_inv16 = consts.tile([128, 1], F32, tag="c_inv16")
    nc.vector.memset(c_inv16, INV16)
    c_exps = consts.tile([128, 1], F32, tag="c_exps")
    nc.vector.memset(c_exps, EXP_SCALE)
    c_nbias = consts.tile([128, 1], F32, tag="c_nbias")
    nc.vector.memset(c_nbias, EXP_BIAS)
    c_heps = consts.tile([128, 1], F32, tag="c_heps")
    nc.vector.memset(c_heps, float(H) * EPS)
    c_hlnh = consts.tile([128, 1], F32, tag="c_hlnh")
    nc.vector.memset(c_hlnh, 0.5 * float(np.log(H)))
    c_nhalf = consts.tile([128, 1], F32, tag="c_nhalf")
    nc.vector.memset(c_nhalf, -0.5)
    c_inv8 = consts.tile([128, 1], F32, tag="c_inv8")
    nc.vector.memset(c_inv8, 0.125)

    def bcast(vec, tg):
        t = consts.tile([128, H], F32, tag=tg)
        v = vec.ap()
        nc.gpsimd.dma_start(
            out=t, in_=bass.AP(tensor=v.tensor, offset=v.offset, ap=[[0, 128]] + list(v.ap)))
        return t

    if not ZERO_AFFINE2:
        g2_b = bcast(g2v, "g2b")
        be2_b = bcast(be2v, "be2b")

    # ---- weights (fp8 except wo2) ----
    wq_sb = wpool.tile([128, HC, H], F8, tag="wq")
    wk_sb = wpool.tile([128, HC, H], F8, tag="wk")
    wv_sb = wpool.tile([128, HC, H], F8, tag="wv")
    wo1_sb = wpool.tile([128, HC, H], F8, tag="wo1")
    wo2_sb = wpool.tile([128, HC, H], BF16, tag="wo2")


    # ---- resident K^T [o, keys], V [keys, o], Q^T [o, q] (all fp8) ----
    k_h = kvq.tile([128, HC, S], F8, tag="k_h")
    v_sb = kvq.tile([128, KC, H], F8, tag="v_sb")
    q_h = kvq.tile([128, HC, NQ], F8, tag="q_h")

    xTk_r = xTk.ap().rearrange("(c p) k -> p c k", p=128)
    xTv_r = xTv.ap().rearrange("(c p) k -> p c k", p=128)

    # ================= phase B: projections (K/V/Q interleaved) =================
    xqT_r = xqT.ap().rearrange("(c p) k -> p c k", p=128)
    with tc.tile_pool(name="psumB", bufs=4, space="PSUM") as psumB:
        # x tiles stream 3 blocks ahead of compute; weights interleave
        xt_q = []

        def load_xt(kb):
            xtv_t = xtp.tile([128, HC, 512], F8, tag="xtv", name=f"xtv_{kb}")
            nc.sync.dma_start(out=xtv_t, in_=xTv_r[:, :, kb * 512:(kb + 1) * 512])
            if ones_mask:
                xtk_t = xtv_t
            else:
                xtk_t = xtp.tile([128, HC, 512], F8, tag="xtk", name=f"xtk_{kb}")
                nc.sync.dma_start(out=xtk_t, in_=xTk_r[:, :, kb * 512:(kb + 1) * 512])
            xt_q.append((xtk_t, xtv_t))

        load_xt(0)
        nc.scalar.dma_start(
            out=wk_sb, in_=wkT.ap().rearrange("(c p) o -> p c o", p=128))
        nc.scalar.dma_start(
            out=wv_sb, in_=wvT.ap().rearrange("(c p) o -> p c o", p=128))
        load_xt(1)
        load_xt(2)
        xq_tiles = []
        for j in range(2):
            xq = xtp.tile([128, HC, 512], F8, tag="xq", name=f"xq_{j}")
            nc.sync.dma_start(out=xq, in_=xqT_r[:, :, j * 512:(j + 1) * 512])
            xq_tiles.append(xq)
        for kb in range(8):
            if kb + 3 < 8:
                load_xt(kb + 3)
            xtk_t, xtv_t = xt_q[kb]
            if kb == 1:
                nc.scalar.dma_start(
                    out=wq_sb, in_=wqT.ap().rearrange("(c p) o -> p c o", p=128))
            elif kb == 6:
                nc.scalar.dma_start(
                    out=wo1_sb, in_=wo1T.ap().rearrange("(c p) o -> p c o", p=128))
            elif kb == 7:
                nc.scalar.dma_start(
                    out=wo2_sb, in_=wo2T.ap().rearrange("(c p) o -> p c o", p=128))

            # K: 3 oc-pairs; psum [0:512]=oc keys, [512:1024]=oc+1 keys (DVE copy)
            for op_ in range(3):
                kps = psumB.tile([128, 1024], F32, tag="kv", name=f"kps_{kb}_{op_}")
                for half in range(2):
                    oc = 2 * op_ + half
                    for i in range(HP):
                        nc.tensor.matmul(
                            kps[:, half * 512:half * 512 + 512],
                            wk_sb[:, 2 * i:2 * i + 2, oc * 128:(oc + 1) * 128],
                            xtk_t[:, 2 * i:2 * i + 2, :],
                            start=(i == 0), stop=(i == HP - 1), perf_mode=DR)
                if ZERO_QKBIAS:
                    nc.vector.tensor_scalar_mul(
                        k_h[:, 2 * op_:2 * op_ + 2, kb * 512:(kb + 1) * 512],
                        kps, INV16)
                else:
                    for half in range(2):
                        oc = 2 * op_ + half
                        nc.vector.tensor_scalar(
                            out=k_h[:, oc, kb * 512:(kb + 1) * 512],
                            in0=kps[:, half * 512:half * 512 + 512],
                            scalar1=INV16, scalar2=bk_sb[:, oc:oc + 1],
                            op0=mybir.AluOpType.mult, op1=mybir.AluOpType.add)

            # V: 4 key tiles of 128; out [k, 768] (ACT copy)
            for ks in range(4):
                vps = psumB.tile([128, 1024], F32, tag="kv", name=f"vps_{kb}_{ks}")
                for i in range(HP):
                    nc.tensor.matmul(vps[:, 0:512],
                                     xtv_t[:, 2 * i:2 * i + 2, ks * 128:(ks + 1) * 128],
                                     wv_sb[:, 2 * i:2 * i + 2, 0:512],
                                     start=(i == 0), stop=(i == HP - 1), perf_mode=DR)
                for i in range(HP):
                    nc.tensor.matmul(vps[:, 512:768],
                                     xtv_t[:, 2 * i:2 * i + 2, ks * 128:(ks + 1) * 128],
                                     wv_sb[:, 2 * i:2 * i + 2, 512:768],
                                     start=(i == 0), stop=(i == HP - 1), perf_mode=DR)
                if ks == 0:
                    nc.vector.tensor_scalar_mul(
                        v_sb[:, kb * 4 + ks, :], vps[:, 0:768], INV16)
                else:
                    nc.scalar.activation(
                        out=v_sb[:, kb * 4 + ks, :], in_=vps[:, 0:768],
                        func=mybir.ActivationFunctionType.Identity, scale=c_inv16)

            # Q: this core's own x columns, two (j, oc) chunks per kb >= 2
            if kb >= 2:
                for t in range(2):
                    idx = (kb - 2) * 2 + t
                    j, oc = idx // HC, idx % HC
                    xq = xq_tiles[j]
                    qps = psumB.tile([128, 1024], F32, tag="kv", name=f"qps_{j}_{oc}")
                    for i in range(HP):
                        nc.tensor.matmul(qps[:, 0:512],
                                         wq_sb[:, 2 * i:2 * i + 2, oc * 128:(oc + 1) * 128],
                                         xq[:, 2 * i:2 * i + 2, :],
                                         start=(i == 0), stop=(i == HP - 1), perf_mode=DR)
                    if ZERO_QKBIAS:
                        nc.scalar.activation(
                            out=q_h[:, oc, j * 512:(j + 1) * 512], in_=qps[:, 0:512],
                            func=mybir.ActivationFunctionType.Identity,
                            scale=c_inv16)
                    else:
                        nc.scalar.activation(
                            out=q_h[:, oc, j * 512:(j + 1) * 512], in_=qps[:, 0:512],
                            func=mybir.ActivationFunctionType.Identity,
                            scale=c_inv16, bias=bq_sb[:, oc:oc + 1])

    if PHASE < 2:
        ctx_mgr.close()
        return

    # ================= attention + tails =================
    psum = ctx_mgr.enter_context(tc.tile_pool(name="psumA", bufs=1, space="PSUM"))

    def ln_stats_fast(pre_src_ps, rs_scale, xbt, pfx):
        """Critical-path variant: stats via DVE bn_stats (shorter chain)."""
        pre = tailp.tile([128, H], F32, tag="pre", name=f"{pfx}pre")
        nc.vector.scalar_tensor_tensor(
            out=pre, in0=pre_src_ps,
            scalar=(1.0 if rs_scale is None else rs_scale), in1=xbt,
            op0=mybir.AluOpType.mult, op1=mybir.AluOpType.add)
        st = smallp.tile([128, 2, 6], F32, tag="bst", name=f"{pfx}bst")
        for i in range(2):
            nc.vector.bn_stats(out=st[:, i, :], in_=pre[:, i * 384:(i + 1) * 384])
        mv = smallp.tile([128, 2], F32, tag="bmv", name=f"{pfx}bmv")
        nc.vector.bn_aggr(out=mv, in_=st)
        lnv = smallp.tile([128, 1], F32, tag="lnv", name=f"{pfx}lnv")
        nc.scalar.activation(out=lnv, in_=mv[:, 1:2],
                             func=mybir.ActivationFunctionType.Ln, bias=c_heps)
        rstd = smallp.tile([128, 1], F32, tag="rstd", name=f"{pfx}rstd")
        nc.scalar.activation(out=rstd, in_=lnv,
                             func=mybir.ActivationFunctionType.Exp,
                             scale=c_nhalf)
        return pre, mv[:, 0:1], rstd

    def ln_stats(pre_src_ps, rs_scale, xbt, pfx, sq_eng="dve"):
        """pre = pre_src*rs + xbt; returns (pre, mu, rstd). The square-sum
        runs on ACT (Square+accum; DVE tensor_tensor_reduce faults on HW)."""
        pre = tailp.tile([128, H], F32, tag="pre", name=f"{pfx}pre")
        s1 = smallp.tile([128, 1], F32, tag="s1", name=f"{pfx}s1")
        nc.vector.scalar_tensor_tensor(
            out=pre, in0=pre_src_ps,
            scalar=(1.0 if rs_scale is None else rs_scale), in1=xbt,
            op0=mybir.AluOpType.mult, op1=mybir.AluOpType.add, accum_out=s1)
        mu = smallp.tile([128, 1], F32, tag="mu", name=f"{pfx}mu")
        nc.vector.tensor_scalar_mul(mu, s1, 1.0 / H)
        s1sq = smallp.tile([128, 1], F32, tag="s1sq", name=f"{pfx}s1sq")
        nc.vector.tensor_mul(out=s1sq, in0=s1, in1=s1)
        sq = tailp.tile([128, H], F32, tag="sq", name=f"{pfx}sq")
        s2 = smallp.tile([128, 1], F32, tag="s2", name=f"{pfx}s2")
        nc.scalar.activation(out=sq, in_=pre,
                             func=mybir.ActivationFunctionType.Square,
                             accum_out=s2)
        hvar = smallp.tile([128, 1], F32, tag="hvar", name=f"{pfx}hvar")
        nc.vector.tensor_scalar(out=hvar, in0=s1sq, scalar1=-1.0 / H, scalar2=s2,
                                op0=mybir.AluOpType.mult, op1=mybir.AluOpType.add)
        lnv = smallp.tile([128, 1], F32, tag="lnv", name=f"{pfx}lnv")
        nc.scalar.activation(out=lnv, in_=hvar,
                             func=mybir.ActivationFunctionType.Ln, bias=c_heps)
        rstd = smallp.tile([128, 1], F32, tag="rstd", name=f"{pfx}rstd")
        nc.scalar.activation(out=rstd, in_=lnv,
                             func=mybir.ActivationFunctionType.Exp,
                             scale=c_nhalf, bias=c_hlnh)
        return pre, mu, rstd

    def ln_apply(pre, mu, rstd, out_tile):
        nc.vector.tensor_scalar(out=out_tile, in0=pre, scalar1=mu, scalar2=rstd,
                                op0=mybir.AluOpType.subtract, op1=mybir.AluOpType.mult)

    def make_pv_emitters(pt_t, qb):
        """PV + den matmuls for query block qb (hc-major so 2 hc accumulators
        share a PSUM bank sequentially). Bank 0 is copied out mid-stream and
        then reused for the den accumulators, freeing a PSUM bank."""
        ctxT_banks = [psum.tile([128, 512], F32, tag="ctxT", bufs=3,
                                name=f"ctxT_{qb}_{b3}") for b3 in range(3)]
        ctx_f8 = tailp.tile([128, HC, QB], F8, tag="ctx", name=f"ctx_{qb}")
        ems = []
        for hc in range(HC):
            bank = ctxT_banks[hc // 2]
            half = (hc % 2) * 256
            for pr in range(KP):
                def em(hc=hc, pr=pr, bank=bank, half=half):
                    nc.tensor.matmul(
                        bank[:, half:half + 256],
                        v_sb[:, 2 * pr:2 * pr + 2, hc * 128:(hc + 1) * 128],
                        pt_t[:, 2 * pr:2 * pr + 2, :],
                        start=(pr == 0), stop=(pr == KP - 1), perf_mode=DR)
                ems.append(em)
            if hc % 2 == 1:
                def em_cp(b3=hc // 2):
                    nc.vector.tensor_scalar_mul(
                        ctx_f8[:, 2 * b3:2 * b3 + 2, :], ctxT_banks[b3], 0.125)
                ems.append(em_cp)
        den_ps = ctxT_banks[0]
        for qs in range(2):
            for pr in range(KP):
                def em(qs=qs, pr=pr):
                    nc.tensor.matmul(
                        den_ps[:, qs:qs + 1],
                        pt_t[:, 2 * pr:2 * pr + 2, qs * 128:(qs + 1) * 128],
                        ones16,
                        start=(pr == 0), stop=(pr == KP - 1), perf_mode=DR)
                ems.append(em)
        return ems, (ctxT_banks, ctx_f8), den_ps

    def emit_tail_a(qb, banks_ctx, den_ps):
        """rs, Wo1, LN1 -> h1 bf16 tiles (ctx copies ride the PV stream)."""
        ctxT_banks, ctx_f8 = banks_ctx
        rs = smallp.tile([128, 2], F32, tag="rs", name=f"rs_{qb}")
        nc.vector.reciprocal(rs, den_ps[:, 0:2])
        xbts = []
        for qs in range(2):
            t0 = qb * QB + qs * 128
            xbt = tailp.tile([128, H], F32, tag="xbt1", name=f"xbt1_{t0}")
            nc.gpsimd.dma_start(out=xbt, in_=xb1.ap()[t0:t0 + 128, :])
            xbts.append(xbt)
        tps_l = []
        for qs in range(2):
            t0 = qb * QB + qs * 128
            tps = psum.tile([128, H], F32, tag="tail", bufs=1, name=f"wo1ps_{t0}")
            for i in range(HP):
                nc.tensor.matmul(tps[:, 0:512],
                                 ctx_f8[:, 2 * i:2 * i + 2, qs * 128:(qs + 1) * 128],
                                 wo1_sb[:, 2 * i:2 * i + 2, 0:512],
                                 start=(i == 0), stop=(i == HP - 1), perf_mode=DR)
            for i in range(HP):
                nc.tensor.matmul(tps[:, 512:768],
                                 ctx_f8[:, 2 * i:2 * i + 2, qs * 128:(qs + 1) * 128],
                                 wo1_sb[:, 2 * i:2 * i + 2, 512:768],
                                 start=(i == 0), stop=(i == HP - 1), perf_mode=DR)
            tps_l.append(tps)
        stats = []
        for qs in range(2):
            t0 = qb * QB + qs * 128
            stats.append(ln_stats_fast(tps_l[qs], rs[:, qs:qs + 1], xbts[qs],
                                       f"a{t0}_"))
        h1s = []
        for qs in range(2):
            t0 = qb * QB + qs * 128
            h1 = tailp.tile([128, H], BF16, tag="h1", bufs=6, name=f"h1_{t0}")
            ln_apply(*stats[qs], h1)
            h1s.append(h1)
        return h1s

    def emit_tail_b(qb, h1s):
        """h1 transpose, Wo2, LN2, (affine,) store — staged thunks."""
        ems = []
        h1Ts = []
        for qs in range(2):
            t0 = qb * QB + qs * 128
            h1 = h1s[qs]
            h1T = tailp.tile([128, HC, 128], BF16, tag="h1T", bufs=4,
                             name=f"h1T_{t0}")
            h1Ts.append(h1T)

            def _mk_tp(hc, h1=h1, h1T=h1T, t0=t0):
                def em():
                    tpp = psum.tile([128, 128], BF16, tag="sb", bufs=3,
                                    name=f"tp_{t0}_{hc}")
                    nc.tensor.transpose(tpp, h1[:, hc * 128:(hc + 1) * 128], ident)
                    if hc % 2 == 0:
                        nc.scalar.activation(
                            out=h1T[:, hc, :], in_=tpp,
                            func=mybir.ActivationFunctionType.Identity)
                    else:
                        nc.vector.tensor_copy(out=h1T[:, hc, :], in_=tpp)
                return em
            for hc in range(HC):
                ems.append(_mk_tp(hc))

        if os.environ.get("KERNEL_ABLATE") == "wo2":
            return ems
        state = {}

        def _mk_wo2mm(qs):
            t0 = qb * QB + qs * 128
            h1T = h1Ts[qs]
            def em():
                xbt = tailp.tile([128, H], F32, tag="xbt2", name=f"xbt2_{t0}")
                nc.gpsimd.dma_start(out=xbt, in_=xb2.ap()[t0:t0 + 128, :])
                state[("xb", qs)] = xbt
                tps = psum.tile([128, H], F32, tag="tail", bufs=1,
                                name=f"wo2ps_{t0}")
                for hc in range(HC):
                    nc.tensor.matmul(tps[:, 0:512], h1T[:, hc, :],
                                     wo2_sb[:, hc, 0:512],
                                     start=(hc == 0), stop=(hc == HC - 1))
                for hc in range(HC):
                    nc.tensor.matmul(tps[:, 512:768], h1T[:, hc, :],
                                     wo2_sb[:, hc, 512:768],
                                     start=(hc == 0), stop=(hc == HC - 1))
                state[("ps", qs)] = tps
            return em

        def _mk_stats(qs):
            t0 = qb * QB + qs * 128
            def em():
                state[("st", qs)] = ln_stats_fast(
                    state[("ps", qs)], None, state[("xb", qs)], f"b{t0}_")
            return em

        def _mk_apply(qs):
            t0 = qb * QB + qs * 128
            def em():
                outt = tailp.tile([128, H], F32, tag="outt", name=f"outt_{t0}")
                if ZERO_AFFINE2:
                    ln_apply(*state[("st", qs)], outt)
                else:
                    norm = tailp.tile([128, H], F32, tag="norm",
                                      name=f"norm_{t0}")
                    ln_apply(*state[("st", qs)], norm)
                    nc.gpsimd.tensor_mul(out=norm, in0=norm, in1=g2_b)
                    nc.gpsimd.tensor_add(out=outt, in0=norm, in1=be2_b)
                nc.sync.dma_start(out=out.ap()[t0:t0 + 128, :], in_=outt)
            return em

        ems.append(_mk_wo2mm(0))
        ems.append(_mk_wo2mm(1))
        ems.append(_mk_stats(0))
        ems.append(_mk_stats(1))
        ems.append(_mk_apply(0))
        ems.append(_mk_apply(1))
        return ems

    DEBUG = os.environ.get("KERNEL_DEBUG")
    DEBUG = os.environ.get("KERNEL_DEBUG")
    DEBUG = os.environ.get("KERNEL_DEBUG")
    DEBUG = os.environ.get("KERNEL_DEBUG")

    prev_pv = None       # (pt tile, qb) awaiting PV
    pend_b = None        # (qb, h1s) awaiting tailB
    self_pv = None       # (banks_ctx, den_ps) when PV already ran in-round
    pt_t = None
    for qb in range(NQB + 2):
        pv_ems = []
        banks_ctx = den_ps = None
        if prev_pv is not None:
            if self_pv is not None:
                banks_ctx, den_ps = self_pv
                self_pv = None
            else:
                pt_prev, qb_prev = prev_pv
                pv_ems, banks_ctx, den_ps = make_pv_emitters(pt_prev, qb_prev)
        tb_ems = []
        if PHASE >= 3 and pend_b is not None:
            if os.environ.get("KERNEL_ABLATE") != "tailb":
                tb_ems = emit_tail_b(*pend_b)
            pend_b = None
        kpv = 0
        ktb = 0
        tail_a_done = False
        if qb < NQB:
            pt_t = ptp.tile([128, KC, QB], F8, tag="pt", name=f"pt_{qb}")
            pv3_ems = None
            for p in range(KP):
                sps = psum.tile([128, 512], F32, tag="sb", bufs=3,
                                name=f"sps_{qb}_{p}")
                for half in range(2):
                    kc = 2 * p + half
                    for i in range(HP):
                        nc.tensor.matmul(
                            sps[:, half * 256:half * 256 + 256],
                            k_h[:, 2 * i:2 * i + 2, kc * 128:(kc + 1) * 128],
                            q_h[:, 2 * i:2 * i + 2, qb * QB:(qb + 1) * QB],
                            start=(i == 0), stop=(i == HP - 1), perf_mode=DR)
                nc.scalar.activation(
                    out=pt_t[:, 2 * p:2 * p + 2, :], in_=sps,
                    func=mybir.ActivationFunctionType.Exp,
                    scale=c_exps, bias=c_nbias)
                while kpv < min(len(pv_ems),
                                (p + 1) * len(pv_ems) // PVD):
                    pv_ems[kpv]()
                    kpv += 1
                if p >= TB0:
                    while ktb < (p - TB0 + 1) * len(tb_ems) // (KP - TB0):
                        tb_ems[ktb]()
                        ktb += 1

        while kpv < len(pv_ems):
            pv_ems[kpv]()
            kpv += 1
        while ktb < len(tb_ems):
            tb_ems[ktb]()
            ktb += 1


        if DEBUG and qb == 1:
            dbg = nc.dram_tensor("dbg_k", [128, H], F8, kind="ExternalOutput")
            nc.sync.dma_start(out=dbg.ap(), in_=k_h[:, :, 0:128])
            dbg2 = nc.dram_tensor("dbg_q", [128, H], F8, kind="ExternalOutput")
            nc.sync.dma_start(out=dbg2.ap(), in_=q_h[:, :, 0:128])
            dbg3 = nc.dram_tensor("dbg_pt", [128, 1024], F8, kind="ExternalOutput")
            nc.sync.dma_start(out=dbg3.ap(), in_=pt_prev[:, 0:4, :])
            dbg4 = nc.dram_tensor("dbg_den", [128, 2], F32, kind="ExternalOutput")
            den_sb = tailp.tile([128, 2], F32, tag="dbgden", name="dbgden")
            nc.vector.tensor_copy(out=den_sb, in_=den_ps[:, 0:2])
            nc.sync.dma_start(out=dbg4.ap(), in_=den_sb)
            dbg5 = nc.dram_tensor("dbg_ctx", [128, 512], F32, kind="ExternalOutput")
            ctx_sb = tailp.tile([128, 512], F32, tag="dbgctx", name="dbgctx")
            nc.vector.tensor_copy(out=ctx_sb, in_=banks_ctx[0][1])
            nc.sync.dma_start(out=dbg5.ap(), in_=ctx_sb)
            dbg6 = nc.dram_tensor("dbg_v", [128, H], F8, kind="ExternalOutput")
            nc.sync.dma_start(out=dbg6.ap(), in_=v_sb[:, 0, :])

        if PHASE >= 3 and prev_pv is not None and not tail_a_done:
            h1s = emit_tail_a(prev_pv[1], banks_ctx, den_ps)
            pend_b = (prev_pv[1], h1s)
        prev_pv = (pt_t, qb) if qb < NQB else None

    ctx_mgr.close()


_CACHE = {}


def _build(zero_qkbias=True, zero_affine2=True, ones_mask=True):
    key = ("nc", zero_qkbias, zero_affine2, ones_mask)
    if key in _CACHE:
        return _CACHE[key]
    nc = bacc.Bacc("TRN2", target_bir_lowering=False, debug=False,
                   enable_asserts=False, num_devices=NCORES)
    io = (
        nc.dram_tensor("xTk", [H, S], F8, kind="ExternalInput"),
        nc.dram_tensor("xTv", [H, S], F8, kind="ExternalInput"),
        nc.dram_tensor("xqT", [H, NQ], F8, kind="ExternalInput"),
        nc.dram_tensor("wqT", [H, H], F8, kind="ExternalInput"),
        nc.dram_tensor("wkT", [H, H], F8, kind="ExternalInput"),
        nc.dram_tensor("wvT", [H, H], F8, kind="ExternalInput"),
        nc.dram_tensor("wo1T", [H, H], F8, kind="ExternalInput"),
        nc.dram_tensor("wo2T", [H, H], BF16, kind="ExternalInput"),
        nc.dram_tensor("bq", [H], F32, kind="ExternalInput"),
        nc.dram_tensor("bk", [H], F32, kind="ExternalInput"),
        nc.dram_tensor("g2", [H], F32, kind="ExternalInput"),
        nc.dram_tensor("be2", [H], F32, kind="ExternalInput"),
        nc.dram_tensor("xb1", [NQ, H], F32, kind="ExternalInput"),
        nc.dram_tensor("xb2", [NQ, H], F32, kind="ExternalInput"),
        nc.dram_tensor("out", [NQ, H], F32, kind="ExternalOutput"),
    )
    with tile.TileContext(nc) as tc:
        _emit(nc, tc, io, zero_qkbias=zero_qkbias, zero_affine2=zero_affine2,
              ones_mask=ones_mask)
    nc.compile()
    _CACHE[key] = nc
    return nc


def _f8(a):
    return np.clip(np.asarray(a, np.float32), -240.0, 240.0).astype(
        ml_dtypes.float8_e4m3)


def kernel(hidden_states, attention_mask, Wq, bq, Wk, bk, Wv, bv,
           Wo1, bo1, g1, beta1, Wo2, bo2, g2, beta2):
    from concourse.bass_utils import run_bass_kernel_spmd

    bf = ml_dtypes.bfloat16
    x = np.asarray(hidden_states, np.float32)
    mask = np.asarray(attention_mask, np.float32)
    Wq32 = np.asarray(Wq, np.float32)
    Wk32 = np.asarray(Wk, np.float32)
    Wv32 = np.asarray(Wv, np.float32)
    Wo132 = np.asarray(Wo1, np.float32)
    Wo232 = np.asarray(Wo2, np.float32)
    g1v = np.asarray(g1, np.float32)
    bv32 = np.asarray(bv, np.float32)

    shared = {
        "wqT": _f8(Wq32.T * 16.0),
        "wkT": _f8(Wk32.T * 16.0),
        "wvT": _f8(Wv32.T * 16.0),
        "wo1T": _f8(Wo132.T * 16.0),
        "wo2T": np.ascontiguousarray(Wo232.T * g1v[:, None]).astype(bf),
        "bq": np.asarray(bq, np.float32), "bk": np.asarray(bk, np.float32),
        "g2": np.asarray(g2, np.float32), "be2": np.asarray(beta2, np.float32),
    }
    # bv folds into xb1: ctx_true = ctxU*rs + bv  ->  + (Wo1 @ bv)
    bv_fold = Wo132 @ bv32
    beta1_fold = np.asarray(beta1, np.float32) @ np.ascontiguousarray(Wo232.T)

    in_maps = []
    for c in range(NCORES):
        b, qc = c // 4, c % 4
        xb = x[b]                                    # [S, H]
        xk = xb * mask[b, 0][:, None]                # mask folded into K input
        chunk = xb[qc * NQ:(qc + 1) * NQ]            # [NQ, H]
        m = {
            "xTk": _f8(np.ascontiguousarray(xk.T)),
            "xTv": _f8(np.ascontiguousarray(xb.T)),
            "xqT": _f8(np.ascontiguousarray(chunk.T)),
            "xb1": (chunk + np.asarray(bo1, np.float32) + bv_fold).astype(np.float32),
            "xb2": (chunk + np.asarray(bo2, np.float32) + beta1_fold).astype(np.float32),
        }
        m.update(shared)
        in_maps.append(m)

    zero_qkbias = bool(
        not np.any(np.asarray(bq, np.float32))
        and not np.any(np.asarray(bk, np.float32)))
    zero_affine2 = bool(
        np.all(np.asarray(g2, np.float32) == 1.0)
        and not np.any(np.asarray(beta2, np.float32)))
    ones_mask = bool(np.all(mask == 1.0))
    nc = _build(zero_qkbias=zero_qkbias, zero_affine2=zero_affine2,
                ones_mask=ones_mask)
    res = run_bass_kernel_spmd(nc, in_maps, core_ids=list(range(NCORES)))
    out = np.empty((B, S, H), np.float32)
    for c in range(NCORES):
        b, qc = c // 4, c % 4
        out[b, qc * NQ:(qc + 1) * NQ] = res.results[c]["out"]
    return out

